# revision 29
# baseline (speedup 1.0000x reference)
"""Single-head causal attention (B=4, S=4096, E=512, DK=DV=64) on 8 trn2 cores.

Sharding: 2 cores per batch element, balanced causal split at 256-row
granularity. Each core owns 8 q-groups of 256 rows at positions
[512g+256, 512g+512). Role A (even cores) holds emb in natural token order so
those positions are its own q rows; role B's host-side embT swaps the two
256-halves of every 512-token block, which lands its q rows (orig
[512g, 512g+256)) at the same static positions while keeping every causal key
inside the group's static 4(g+1)-key-tile prefix. Both roles run the identical
SPMD program; the only per-role differences are host data (embT layout, mask
slab for the last quad's first two tiles: keep for A, zero for B).

Everything is computed transposed (d-major) until PV. Scores use fp8e4m3
DoubleRow matmuls (2x PE throughput; the second contraction row is zero).
PV is "flipped": the exp'd score tile P^T is the stationary operand and the
ones-augmented V (fp8, token-natural) is the moving operand, so the output
comes out token-major — no finalize transposes — and each 256-key pair costs
one 66-wide DoubleRow matmul. Group 0 (rows < 512, where softmax mass
concentrates on few keys) runs PV in bf16 for precision. The softmax column
sum rides along as a ones column of V; exp runs on the Activation engine in
[128, 1024] quads; triangle mask multiplies run on the otherwise-idle GPSIMD
engine.
"""

import sys

for _p in ("/opt/trn_rl_repo",):
    if _p not in sys.path:
        sys.path.insert(0, _p)

import numpy as np
import ml_dtypes

import concourse.bass as bass
import concourse.bacc as bacc
import concourse.mybir as mybir
from concourse.bass_utils import run_bass_kernel_spmd
from concourse.tile import TileContext

B, S, E, DK, DV = 4, 4096, 512, 64, 64
P = 128
NCORES = 8
NG = 8          # attention q-groups per core
QG = 256        # q rows per group
KG = 512        # tokens per kv/projection group
NJ = S // KG    # kv groups (8)
EC = E // P     # embedding chunks (4)
F32 = mybir.dt.float32
BF16 = mybir.dt.bfloat16
F8 = mybir.dt.float8e4
DR = mybir.MatmulPerfMode.DoubleRow
VW = DV + 2     # V columns + ones column + pad (66)


def build_program():
    nc = bacc.Bacc("TRN2", target_bir_lowering=False, debug=False, num_devices=NCORES)

    embT = nc.declare_dram_parameter("embT", [E, S], BF16, isOutput=False)
    # cols 0:64 Wk, 64:128 Wv, 128:192 Wq, 192:256 zero pad
    wqkv = nc.declare_dram_parameter("wqkv", [E, 2 * P], BF16, isOutput=False)
    # col 0: [bk; bv], col 1 rows 0:64: bq
    bkv = nc.declare_dram_parameter("bkv", [P, 2], F32, isOutput=False)
    # cols 0:1024 mask (last-quad slab), 1024:1152 identity
    cst = nc.declare_dram_parameter("cst", [P, 4 * QG + P], BF16, isOutput=False)
    out = nc.declare_dram_parameter("out", [NG, QG, DV], F32, isOutput=True)

    with TileContext(nc) as tc:
        with (
            tc.tile_pool(name="singles", bufs=1) as singles,
            tc.tile_pool(name="pt", bufs=6) as pt_pool,
            tc.tile_pool(name="pt0", bufs=1) as pt0_pool,
            tc.tile_pool(name="fin", bufs=2) as fin_pool,
            tc.tile_pool(name="res", bufs=2) as res_pool,
            tc.tile_pool(name="ps_s", bufs=2, space="PSUM") as ps_pool,
            tc.tile_pool(name="ps_o", bufs=1, space="PSUM") as po_pool,
            tc.tile_pool(name="ps_sm", bufs=2, space="PSUM") as sm_pool,
        ):
            # ---- persistent tensors ----
            et = [
                singles.tile([P, EC, KG], BF16, name=f"et{j}") for j in range(NJ)
            ]
            k8 = singles.tile([DK, 2, S], F8)            # K^T, j=1 zeros
            q8 = singles.tile([DK, 2, NG * QG], F8)      # Q^T, j=1 zeros
            vt = singles.tile([P, S], BF16)              # rows 64:128 = V^T
            vn8 = singles.tile([P, S // P, VW], F8)      # V natural + ones + pad
            vnb = singles.tile([P, 4, VW], BF16)         # bf16 V for group 0
            w_sb = singles.tile([P, EC, 2 * P], BF16)
            bkv_sb = singles.tile([P, 2], F32)
            cst_sb = singles.tile([P, 4 * QG + P], BF16)
            msk_sb = cst_sb[:, 0:4 * QG]
            idb_sb = cst_sb[:, 4 * QG:4 * QG + P]
            po = po_pool.tile([P, 2, KG], F32)           # PV accum, h-halves in
                                                         # separate PSUM banks

            # ---- input DMAs, in consumption order ----
            embT_r = embT[:].rearrange("(c p) t -> p c t", p=P)

            def et_dma(j, lo=0, hi=KG):
                nc.sync.dma_start(
                    out=et[j][:, :, lo:hi], in_=embT_r[:, :, KG * j + lo:KG * j + hi]
                )

            # biases ride the Act queue so they don't cost an SP/HWDGE slot
            # ahead of the embedding stream
            nc.scalar.dma_start(out=bkv_sb, in_=bkv[:])
            nc.sync.dma_start(out=w_sb, in_=wqkv[:].rearrange("(c p) m -> p c m", p=P))
            # token-split of et0: the second half feeds q_proj(0), so the
            # first scores quad's projection chain starts one transfer earlier
            et_dma(0, QG, KG)
            et_dma(0, 0, QG)
            et_dma(1)
            nc.sync.dma_start(out=cst_sb, in_=cst[:])
            et_dma(2)
            et_dma(3)
            et_dma(4)
            et_dma(5)
            et_dma(6)
            et_dma(7)

            # DoubleRow j=1 contraction rows are zero; GPSIMD is idle in the
            # prologue and its memsets keep these off the DMA/DVE paths
            nc.gpsimd.memset(k8[:, 1, :], 0.0)
            nc.gpsimd.memset(q8[:, 1, :], 0.0)
            nc.vector.memset(vn8[:, :, DV:DV + 1], 1.0)
            nc.vector.memset(vn8[:, :, DV + 1:DV + 2], 0.0)
            nc.vector.memset(vnb[:, :, DV:DV + 1], 1.0)
            nc.vector.memset(vnb[:, :, DV + 1:DV + 2], 0.0)

            # ---- building blocks ----
            def kv_proj(j, lo=0, hi=KG):
                pkv = sm_pool.tile([P, KG], F32, tag="sm")
                for c in range(EC):
                    nc.tensor.matmul(
                        pkv[:, lo:hi], w_sb[:, c, 0:P], et[j][:, c, lo:hi],
                        start=(c == 0), stop=(c == EC - 1),
                    )
                nc.vector.tensor_scalar_add(
                    k8[:, 0, KG * j + lo:KG * j + hi], pkv[0:DK, lo:hi],
                    bkv_sb[0:DK, 0:1]
                )
                nc.vector.tensor_scalar_add(
                    vt[DK:P, KG * j + lo:KG * j + hi], pkv[DK:P, lo:hi],
                    bkv_sb[DK:P, 0:1]
                )

            def vnat(j):
                pv = sm_pool.tile([P, 4, DV], BF16, tag="sm")
                for s in range(4):
                    nc.tensor.transpose(
                        pv[:, s, :],
                        vt[DK:P, (4 * j + s) * P:(4 * j + s + 1) * P],
                        idb_sb[DK:P, DK:P],
                    )
                nc.vector.tensor_copy(vn8[:, 4 * j:4 * j + 4, 0:DV], pv)
                if j == 0:
                    nc.vector.tensor_copy(vnb[:, 0:4, 0:DV], pv)

            def q_proj(g):
                pq = sm_pool.tile([DK, QG], F32, tag="sm")
                for c in range(EC):
                    nc.tensor.matmul(
                        pq, w_sb[:, c, P:P + DK], et[g][:, c, QG:2 * QG],
                        start=(c == 0), stop=(c == EC - 1),
                    )
                nc.vector.tensor_scalar_add(
                    q8[:, 0, QG * g:QG * (g + 1)], pq, bkv_sb[0:DK, 1:2]
                )

            def scores(g, q):
                ps = ps_pool.tile([P, 4, QG], F32, tag="ps")
                qs = q8[:, :, QG * g:QG * (g + 1)]
                for i in range(4):
                    kt = 4 * q + i
                    nc.tensor.matmul(
                        ps[:, i, :], k8[:, :, kt * P:(kt + 1) * P], qs,
                        start=True, stop=True, perf_mode=DR,
                    )
                return ps

            def attention(g, fillers=None, first_ps=None):
                fillers = dict(fillers or {})
                n_q = g + 1
                # the masked quad (q == g) runs second: its GPSIMD mask-mul
                # overlaps later quads' exp instead of sitting in the tail
                order = [0, g] + list(range(1, g)) if g > 0 else [0]
                ps_cur = first_ps if first_ps is not None else scores(g, order[0])
                nxt = None
                pending_pv = []
                for i, q in enumerate(order):
                    if g == 0:
                        # two halves: the first needs only the first
                        # half-projection, starting the Act spine earlier
                        pt = pt0_pool.tile([P, 4, QG], BF16, tag="pt0")
                        for z in range(2):
                            nc.scalar.activation(
                                pt[:, 2 * z:2 * z + 2, :],
                                ps_cur[:, 2 * z:2 * z + 2, :],
                                mybir.ActivationFunctionType.Exp, scale=0.125,
                            )
                    else:
                        pt = pt_pool.tile([P, 4, QG], F8, tag="pt")
                        nc.scalar.activation(
                            pt, ps_cur, mybir.ActivationFunctionType.Exp,
                            scale=0.125,
                        )
                    if i + 1 < n_q:
                        ps_cur = scores(g, order[i + 1])
                        for f in fillers.pop(i, ()):
                            f()
                    else:
                        # last slot: q_proj first (the hoisted scores reads its
                        # q8 slice — emission order is a correctness
                        # requirement), then the hoist, then the bulkier
                        # kv-side fillers so they don't delay the next group's
                        # first exp
                        for f in fillers.pop("pre", ()):
                            f()
                        if g + 1 < NG:
                            nxt = scores(g + 1, 0)
                        for f in fillers.pop("post", ()):
                            f()
                    if q == g:
                        # DVE, h-split: GPSIMD's queue suffers multi-us Drain
                        # holds that start masks late and stall the PV chain
                        for h in range(2):
                            nc.vector.tensor_mul(
                                pt[:, :, P * h:P * (h + 1)],
                                pt[:, :, P * h:P * (h + 1)],
                                msk_sb.rearrange("p (a b) -> p a b", b=QG)[
                                    :, :, P * h:P * (h + 1)
                                ],
                            )

                    def mk_pv(pt, q, i):
                        def emit():
                            if g == 0:
                                for kt in range(4):
                                    for h in range(2):
                                        nc.tensor.matmul(
                                            po[:, h, 0:VW],
                                            pt[:, kt, P * h:P * (h + 1)],
                                            vnb[:, kt, :],
                                            start=(kt == 0), stop=(kt == 3),
                                        )
                            else:
                                for p2 in range(2):
                                    for h in range(2):
                                        nc.tensor.matmul(
                                            po[:, h, 0:VW],
                                            pt[:, 2 * p2:2 * p2 + 2,
                                               P * h:P * (h + 1)],
                                            vn8[:, 2 * (2 * q + p2):
                                                2 * (2 * q + p2) + 2, :],
                                            start=(i == 0 and p2 == 0),
                                            stop=(i == n_q - 1 and p2 == 1),
                                            perf_mode=DR,
                                        )
                        return emit

                    # defer PV one position: a PV quad parked on its pt fills
                    # the 4-deep PE wait queue (Ldweights+Matmult pairs) and
                    # head-of-line-blocks the next scores the Act engine needs
                    pending_pv.append(mk_pv(pt, q, i))
                    if len(pending_pv) > 2:
                        pending_pv.pop(0)()
                for i in sorted(fillers):
                    for f in fillers[i]:
                        f()
                for f in pending_pv:
                    f()
                rs = fin_pool.tile([P, 2, 1], F32, tag="rs")
                nc.vector.reciprocal(rs, po[:, :, DV:DV + 1])
                res = res_pool.tile([P, 2, DV], F32, tag="res")
                for h in range(2):
                    nc.vector.tensor_scalar_mul(
                        res[:, h, :], po[:, h, 0:DV], rs[:, h, :]
                    )
                nc.sync.dma_start(
                    out=out[:][g].rearrange("(s p) d -> p s d", p=P), in_=res
                )
                return nxt

            # ---- emission schedule ----
            # q_proj first: it only needs the first-landed et0 half. Group 0's
            # first scores pair interleaves with the kv_proj halves so its
            # first exp-half starts as early as possible.
            q_proj(0)
            kv_proj(0, 0, QG)
            ps0 = ps_pool.tile([P, 4, QG], F32, tag="ps")
            qs0 = q8[:, :, 0:QG]
            for i in range(2):
                nc.tensor.matmul(
                    ps0[:, i, :], k8[:, :, i * P:(i + 1) * P], qs0,
                    start=True, stop=True, perf_mode=DR,
                )
            kv_proj(0, QG, KG)
            for i in range(2, 4):
                nc.tensor.matmul(
                    ps0[:, i, :], k8[:, :, i * P:(i + 1) * P], qs0,
                    start=True, stop=True, perf_mode=DR,
                )
            vnat(0)

            def KP(j):
                return lambda: kv_proj(j)

            def VN(j):
                return lambda: vnat(j)

            def QP(g):
                return lambda: q_proj(g)

            nxt = ps0
            for g in range(NG):
                fillers = {}
                if g + 1 < NG:
                    fillers["pre"] = [QP(g + 1)]
                    fillers["post"] = [KP(g + 1), VN(g + 1)]
                nxt = attention(g, fillers, first_ps=nxt)

    nc.compile()
    return nc


_PROGRAM = None


def _get_program():
    global _PROGRAM
    if _PROGRAM is None:
        _PROGRAM = build_program()
    return _PROGRAM


def _host_inputs(emb, Wq_w, Wq_b, Wk_w, Wk_b, Wv_w, Wv_b):
    bf = ml_dtypes.bfloat16
    wqkv = np.zeros((E, 2 * P), np.float32)
    wqkv[:, 0:DK] = Wk_w
    wqkv[:, DK:2 * DK] = Wv_w
    wqkv[:, P:P + DK] = Wq_w
    wqkv = wqkv.astype(bf)

    bkv = np.zeros((P, 2), np.float32)
    bkv[0:DK, 0] = Wk_b
    bkv[DK:P, 0] = Wv_b
    bkv[0:DK, 1] = Wq_b

    idb = np.eye(P, dtype=np.float32)

    # mask for the last quad [128, 4, 256] -> flat [128, 1024]:
    # tiles 0,1: keep for role A / zero for role B; tiles 2,3: triangles
    pp = np.arange(P)[:, None]
    jj = np.arange(QG)[None, :]
    t0 = (pp <= jj).astype(np.float32)
    t1 = (pp + P <= jj).astype(np.float32)
    cst_by_role = []
    for role in range(2):
        c = np.ones((P, QG), np.float32) if role == 0 else np.zeros((P, QG), np.float32)
        m = np.concatenate([c, c, t0, t1, idb], axis=1).astype(bf)
        cst_by_role.append(m)
    return wqkv, bkv, cst_by_role


def kernel(embedding_matrix, Wq_w, Wq_b, Wk_w, Wk_b, Wv_w, Wv_b):
    emb = np.asarray(embedding_matrix, dtype=np.float32)
    wqkv, bkv, cst_by_role = _host_inputs(
        emb, np.asarray(Wq_w, np.float32), np.asarray(Wq_b, np.float32),
        np.asarray(Wk_w, np.float32), np.asarray(Wk_b, np.float32),
        np.asarray(Wv_w, np.float32), np.asarray(Wv_b, np.float32),
    )
    bf = ml_dtypes.bfloat16

    in_maps = []
    for c in range(NCORES):
        b, role = c // 2, c % 2
        e = emb[b]
        if role == 1:
            # swap the 256-halves of every 512-token block
            e = e.reshape(NJ, 2, QG, E)[:, ::-1].reshape(S, E)
        embT_sw = np.ascontiguousarray(e.T.astype(bf))
        in_maps.append({
            "embT": embT_sw, "wqkv": wqkv, "bkv": bkv,
            "cst": cst_by_role[role],
        })

    nc = _get_program()
    results = run_bass_kernel_spmd(nc, in_maps, list(range(NCORES))).results

    out = np.empty((B, S, DV), np.float32)
    for c in range(NCORES):
        b, role = c // 2, c % 2
        o = results[c]["out"]                    # [NG, 256, 64]
        for g in range(NG):
            q0 = KG * g + (QG if role == 0 else 0)
            out[b, q0:q0 + QG] = o[g]
    return out


if __name__ == "__main__":
    rng = np.random.default_rng(0)
    ins = {
        "embedding_matrix": rng.standard_normal((B, S, E), dtype=np.float32),
        "Wq_w": rng.standard_normal((E, DK), dtype=np.float32) * 0.04,
        "Wq_b": rng.standard_normal((DK,), dtype=np.float32) * 0.04,
        "Wk_w": rng.standard_normal((E, DK), dtype=np.float32) * 0.04,
        "Wk_b": rng.standard_normal((DK,), dtype=np.float32) * 0.04,
        "Wv_w": rng.standard_normal((E, DV), dtype=np.float32) * 0.04,
        "Wv_b": rng.standard_normal((DV,), dtype=np.float32) * 0.04,
    }
    o = kernel(**ins)
    print("kernel ran, out:", o.shape, o.dtype, float(np.abs(o).max()))
